# revision 3
# baseline (speedup 1.0000x reference)
"""Trainium2 Bass kernel for the attention-pooling layer.

Computation (per sample b):
    q = input2 @ fc_w.T + fc_b                      # [B, C1]
    scores[b, p] = <input1[b, :, p], q[b]>          # [B, HW]
    attn = softmax(scores, axis=1)
    out[b, c] = sum_p input1[b, c, p] * attn[b, p]  # [B, C1]

Sharding: data-parallel over batch across 8 NeuronCores (8 samples each).
Default (V2=True) also shards fc_w.T over C2 across the cores: each core
loads a 1MB slice instead of the full 8.4MB, computes partial q for ALL 64
samples over its slice (bias/8 folded in as a K=1 matmul so the sum restores
it), ReduceScatters the [64, 1024] partials so core i receives the summed q
for its own 8 samples, and transposes q on-chip via TensorE.  This cuts
per-core HBM traffic from 34.1MB to 26.8MB (-21%), the roofline for this
memory-bound problem.  Host pre-transposes fc_w and lays out the input2
slices so every device-side DMA is wide-descriptor friendly.

Per-core phases (one TileContext; Tile inserts all semaphores):
  1. q/qT as above (V2=False falls back to replicated-weights direct-qT
     matmuls, no collective).
  2. scores (per sample): M=1 TensorE matmuls accumulating over the 8
     C1-chunks, N=392 halves so each accumulation group stays in one PSUM
     bank; x streams in per-sample halves, 4 samples in flight.
  3. softmax (per sample): one negated reduce_max (DVE) over both PSUM
     halves, one ScalarE Exp whose elementwise output is the unnormalized
     attn row and whose accum_out is its sum, a DVE reciprocal; GpSimd
     (otherwise idle) broadcasts both the attn row and 1/sum across the 128
     partitions.
  4. pooling (per sample, per C1-chunk): one DVE scalar_tensor_tensor
     (x * 1/sum * attn with fused free-dim accum_out) -- the 1/sum rides the
     per-partition scalar slot, so normalization costs nothing extra.
"""

import numpy as np

import concourse.bacc as bacc
import concourse.mybir as mybir
import concourse.tile as tile
from concourse import masks
from concourse.bass_utils import run_bass_kernel_spmd

F32 = mybir.dt.float32

B, C1, C2, HW = 64, 1024, 2048, 784
NCORES = 8
BL = B // NCORES          # samples per core
P = 128                   # partitions
CO = C1 // P              # 8 c1 chunks
KC = C2 // P              # 16 c2 chunks
HH = HW // 2              # 392, half the pixels (fits one PSUM bank)
XH = 2                    # x DMA split: halves of the c1-chunks per sample
COH = CO // XH            # c1-chunks per x half-tile
KL = C2 // NCORES // P    # v2: c2-chunks of fc_w per core
V2 = True                 # shard fc_w over cores + ReduceScatter partial q
SIM_NO_CC = False         # timeline-sim only: replace collective with a DMA

_CACHE = {}


def _build(repeat=1):
    nc = bacc.Bacc("TRN2", target_bir_lowering=False, debug=False)

    x = nc.dram_tensor("x", [BL, C1, HW], F32, kind="ExternalInput").ap()
    fcb = nc.dram_tensor("fcb", [1, C1], F32, kind="ExternalInput").ap()
    out = nc.dram_tensor("out", [P, BL * CO], F32, kind="ExternalOutput").ap()
    if V2:
        wts = nc.dram_tensor("wts", [KL * P, C1], F32, kind="ExternalInput").ap()
        in2ta = nc.dram_tensor("in2ta", [P, KL * B], F32, kind="ExternalInput").ap()
        qin = nc.dram_tensor("qin", [B, C1], F32).ap()
        qout = nc.dram_tensor("qout", [BL, C1], F32).ap()
        with tile.TileContext(nc) as tc:
            _emit_v2(tc, nc, x, wts, in2ta, fcb, qin, qout, out)
    else:
        wt = nc.dram_tensor("wt", [C2, C1], F32, kind="ExternalInput").ap()
        in2t = nc.dram_tensor("in2t", [P, KC * BL], F32, kind="ExternalInput").ap()
        with tile.TileContext(nc) as tc:
            _emit(tc, nc, x, wt, in2t, fcb, out, repeat=repeat)

    nc.compile()
    return nc


def _emit_v2(tc, nc, x, wts, in2ta, fcb, qin, qout, out):
    """fc_w C2-sharded across cores + ReduceScatter of the partial q."""
    import contextlib

    ctx = contextlib.ExitStack()
    with ctx:
        const = ctx.enter_context(tc.tile_pool(name="const", bufs=1))
        xp = ctx.enter_context(tc.tile_pool(name="xp", bufs=6 * XH + 1))
        sm = ctx.enter_context(tc.tile_pool(name="sm", bufs=2))

        in2_sb = const.tile([P, KL * B], F32, name="in2_sb", tag="in2_sb")
        nc.sync.dma_start(out=in2_sb[:], in_=in2ta)
        fcb_sb = const.tile([1, C1], F32, name="fcb_sb", tag="fcb_sb")
        nc.sync.dma_start(out=fcb_sb[:], in_=fcb)
        ones_sb = const.tile([1, P], F32, name="ones_sb", tag="ones_sb")
        nc.vector.memset(ones_sb[:], 1.0)
        oinv_sb = const.tile([1, P], F32, name="oinv_sb", tag="oinv_sb")
        nc.vector.memset(oinv_sb[:], 1.0 / NCORES)
        ident = const.tile([P, P], F32, name="ident", tag="ident")
        masks.make_identity(nc, ident[:])

        wts_sb = const.tile([P, KL, C1], F32, name="wts_sb", tag="wts_sb")
        wtsr = wts.rearrange("(k p) c -> p k c", p=P)
        for kk in range(KL):
            nc.sync.dma_start(
                out=wts_sb[:, kk:kk + 1, :], in_=wtsr[:, kk:kk + 1, :]
            )

        xr = x.rearrange("b (co ci) q -> b ci co q", ci=P)
        x_sb = []
        for b in range(BL):
            halves = []
            for h in range(XH):
                t = xp.tile([P, COH, HW], F32, name="x_sb", tag="x_sb")
                nc.sync.dma_start(
                    out=t[:], in_=xr[b, :, h * COH:(h + 1) * COH, :]
                )
                halves.append(t)
            x_sb.append(halves)

        # partial q for ALL 64 samples over this core's C2 slice
        q_sb = const.tile([B, C1], F32, name="q_sb", tag="q_sb")
        with tc.tile_pool(name="q_pp", bufs=2, space="PSUM") as q_pp:
            for h in range(2):
                q_ps = q_pp.tile([B, 512], F32, name="q_ps", tag="q_ps")
                for kk in range(KL):
                    nc.tensor.matmul(
                        q_ps[:],
                        in2_sb[:, kk * B:(kk + 1) * B],
                        wts_sb[:, kk, h * 512:(h + 1) * 512],
                        start=(kk == 0),
                        stop=False,
                    )
                # bias/8 on every core; the ReduceScatter sum restores it
                nc.tensor.matmul(
                    q_ps[:],
                    oinv_sb[0:1, 0:B],
                    fcb_sb[0:1, h * 512:(h + 1) * 512],
                    start=False,
                    stop=True,
                )
                nc.scalar.copy(
                    out=q_sb[:, h * 512:(h + 1) * 512], in_=q_ps[:]
                )
        nc.scalar.dma_start(out=qin, in_=q_sb[:])
        if SIM_NO_CC:
            nc.scalar.dma_start(out=qout, in_=qin[0:BL, :])
        else:
            nc.gpsimd.collective_compute(
                "ReduceScatter",
                mybir.AluOpType.add,
                replica_groups=[list(range(NCORES))],
                ins=[qin],
                outs=[qout],
            )
        qrow = const.tile([BL, C1], F32, name="qrow", tag="qrow")
        nc.scalar.dma_start(out=qrow[:], in_=qout)

        # transpose q[b, c1] -> qT[ci, j, b] via TensorE
        qt_all = const.tile([P, CO, BL], F32, name="qt_all", tag="qt_all")
        with tc.tile_pool(name="qt_pp", bufs=4, space="PSUM") as qt_pp:
            for j in range(CO):
                qt_ps = qt_pp.tile([P, BL], F32, name="qt_ps", tag="qt_ps")
                nc.tensor.transpose(
                    qt_ps[:], qrow[:, j * P:(j + 1) * P], ident[0:BL, 0:BL]
                )
                nc.scalar.copy(out=qt_all[:, j, :], in_=qt_ps[:])

        s_pp = ctx.enter_context(tc.tile_pool(name="s_pp", bufs=4, space="PSUM"))
        gall_d = const.tile([P, BL * DCO], F32, name="gall_d", tag="gall_d")
        gall_a = const.tile([P, BL * GCO], F32, name="gall_a", tag="gall_a")
        outr = out.rearrange("p (b co) -> p b co", b=BL)
        for b in range(BL):
            _emit_sample(tc, nc, sm, s_pp, x_sb, qt_all, gall_d, gall_a, b,
                         outr=outr)


DCO = 6                   # pooling chunks on DVE (rest via GpSimd+ScalarE)
GCO = CO - DCO


def _emit_sample(tc, nc, sm, s_pp, x_sb, qt_all, gall_d, gall_a, b,
                 outr=None):
    s_ps = s_pp.tile([1, 2, 512], F32, name="s_ps", tag="s_ps")
    for co in range(CO):
        rhs_tile = x_sb[b][co // COH]
        for h in range(2):
            nc.tensor.matmul(
                s_ps[0:1, h, 0:HH],
                qt_all[:, co, b:b + 1],
                rhs_tile[:, co % COH, h * HH:(h + 1) * HH],
                start=(co == 0),
                stop=(co == CO - 1),
            )
    nm = sm.tile([1, 1], F32, name="nm", tag="nm")
    nc.vector.tensor_reduce(
        nm[:], s_ps[0:1, :, 0:HH], axis=mybir.AxisListType.XY,
        op=mybir.AluOpType.max, negate=True,
    )
    l = sm.tile([1, 1], F32, name="l", tag="l")
    ar = sm.tile([1, HW], F32, name="ar", tag="ar")
    nc.scalar.activation(
        ar.rearrange("p (h n) -> p h n", h=2),
        s_ps[0:1, :, 0:HH],
        mybir.ActivationFunctionType.Exp,
        bias=nm[:], accum_out=l[:],
    )
    # a_sb only depends on the Exp output -- broadcast it first so it is
    # not queued on GpSimd behind r_bc's wait for the DVE reciprocal
    a_sb = sm.tile([P, HW], F32, name="a_sb", tag="a_sb")
    nc.gpsimd.partition_broadcast(a_sb[:], ar[:])
    r = sm.tile([1, 1], F32, name="r", tag="r")
    nc.vector.reciprocal(r[:], l[:])
    r_bc = sm.tile([P, 1], F32, name="r_bc", tag="r_bc")
    nc.gpsimd.partition_broadcast(r_bc[:], r[:])

    waste = sm.tile([P, HW], F32, name="waste", tag="waste", bufs=1)
    for co in range(DCO):
        in0 = x_sb[b][co // COH][:, co % COH, :]
        nc.vector.scalar_tensor_tensor(
            out=waste[:], in0=in0, scalar=r_bc[:], in1=a_sb[:],
            op0=mybir.AluOpType.mult, op1=mybir.AluOpType.mult,
            accum_out=gall_d[:, b * DCO + co:b * DCO + co + 1],
        )
    # remaining chunks ride GpSimd (mult) + ScalarE (scaled Copy with fused
    # accumulate); 1/sum applies via the activation's per-partition scale
    wa = sm.tile([P, HW], F32, name="wa", tag="wa", bufs=1)
    for cg in range(GCO):
        co = DCO + cg
        in0 = x_sb[b][co // COH][:, co % COH, :]
        wg = sm.tile([P, HW], F32, name="wg", tag="wg", bufs=2)
        nc.gpsimd.tensor_tensor(
            out=wg[:], in0=in0, in1=a_sb[:], op=mybir.AluOpType.mult
        )
        nc.scalar.activation(
            wa[:], wg[:], mybir.ActivationFunctionType.Copy,
            bias=0.0, scale=r_bc[:],
            accum_out=gall_a[:, b * GCO + cg:b * GCO + cg + 1],
        )
    if outr is not None:
        # stream this sample's pooled output now, on the ACT HWDGE ring:
        # these waits must not stall the SP ring, which carries the x
        # input stream (HWDGE executes FIFO per issuing engine)
        nc.scalar.dma_start(
            out=outr[:, b, 0:DCO],
            in_=gall_d[:, b * DCO:(b + 1) * DCO],
        )
        nc.scalar.dma_start(
            out=outr[:, b, DCO:CO],
            in_=gall_a[:, b * GCO:(b + 1) * GCO],
        )


def _emit(tc, nc, x, wt, in2t, fcb, out, repeat=1):
    import contextlib

    ctx = contextlib.ExitStack()
    with ctx:
        const = ctx.enter_context(tc.tile_pool(name="const", bufs=1))
        wtp = ctx.enter_context(tc.tile_pool(name="wtp", bufs=1))
        xp = ctx.enter_context(tc.tile_pool(name="xp", bufs=2 * XH * 2))
        sm = ctx.enter_context(tc.tile_pool(name="sm", bufs=2))

        # ---- constants / small loads -------------------------------------
        in2t_sb = const.tile([P, KC * BL], F32, name="in2t_sb", tag="in2t_sb")
        nc.sync.dma_start(out=in2t_sb[:], in_=in2t)
        fcb_sb = const.tile([1, C1], F32, name="fcb_sb", tag="fcb_sb")
        nc.sync.dma_start(out=fcb_sb[:], in_=fcb)
        ones_sb = const.tile([1, P], F32, name="ones_sb", tag="ones_sb")
        nc.vector.memset(ones_sb[:], 1.0)

        for rep in range(repeat):
            # ---- big loads ----------------------------------------------------
            # wt[c2, c1] -> [p, k, c1], split so qT accumulation can start
            # while later chunks are still in flight
            wt_sb = wtp.tile([P, KC, C1], F32, name="wt_sb", tag="wt_sb")
            wtr = wt.rearrange("(k p) c -> p k c", p=P)
            WTC = 4
            for wch in range(WTC):
                ks = slice(wch * (KC // WTC), (wch + 1) * (KC // WTC))
                nc.sync.dma_start(out=wt_sb[:, ks, :], in_=wtr[:, ks, :])

            # x[b, (co ci), p] -> per (b, half): [ci, coh, pix]
            xr = x.rearrange("b (co ci) q -> b ci co q", ci=P)
            x_sb = []
            for b in range(BL):
                halves = []
                for h in range(XH):
                    t = xp.tile([P, COH, HW], F32, name="x_sb", tag="x_sb")
                    nc.sync.dma_start(
                        out=t[:], in_=xr[b, :, h * COH:(h + 1) * COH, :]
                    )
                    halves.append(t)
                x_sb.append(halves)

            # ---- phase 1: qT[ci, j, b] ---------------------------------------
            qt_all = const.tile([P, CO, BL], F32, name="qt_all", tag="qt_all")
            with tc.tile_pool(name=f"qt_pp{rep}", bufs=CO, space="PSUM") as qt_pp:
                qt_ps = []
                for j in range(CO):
                    qt_ps.append(qt_pp.tile([P, BL], F32, name="qt_ps", tag="qt_ps"))
                for k in range(KC):
                    for j in range(CO):
                        nc.tensor.matmul(
                            qt_ps[j][:],
                            wt_sb[:, k, j * P:(j + 1) * P],
                            in2t_sb[:, k * BL:(k + 1) * BL],
                            start=(k == 0),
                            stop=False,
                        )
                for j in range(CO):
                    # bias via K=1 matmul: out[m, n] += fcb[j*128+m] * 1
                    nc.tensor.matmul(
                        qt_ps[j][:],
                        fcb_sb[0:1, j * P:(j + 1) * P],
                        ones_sb[0:1, 0:BL],
                        start=False,
                        stop=True,
                    )
                    nc.scalar.copy(out=qt_all[:, j, :], in_=qt_ps[j][:])

            # ---- per-sample: scores -> softmax -> pooled ---------------------
            rep_ctx = contextlib.ExitStack()
            s_pp = rep_ctx.enter_context(
                tc.tile_pool(name=f"s_pp{rep}", bufs=4, space="PSUM"))
            gall_d = const.tile([P, BL * DCO], F32, name="gall_d", tag="gall_d")
            gall_a = const.tile([P, BL * GCO], F32, name="gall_a", tag="gall_a")
            outr = out.rearrange("p (b co) -> p b co", b=BL)
            for b in range(BL):
                _emit_sample(tc, nc, sm, s_pp, x_sb, qt_all, gall_d, gall_a, b,
                             outr=outr)
            rep_ctx.close()


def _get_nc():
    key = ("nc", V2)
    if key not in _CACHE:
        _CACHE[key] = _build()
    return _CACHE[key]


def _in_maps(input1, input2, fc_w, fc_b):
    input1 = np.ascontiguousarray(np.asarray(input1, dtype=np.float32))
    input2 = np.ascontiguousarray(np.asarray(input2, dtype=np.float32))
    fc_w = np.asarray(fc_w, dtype=np.float32)
    fc_b = np.asarray(fc_b, dtype=np.float32)

    wt = np.ascontiguousarray(fc_w.T)                       # [C2, C1]
    fcb = np.ascontiguousarray(fc_b.reshape(1, C1))
    # v2: in2ta[p, kk*B + ball] = input2[ball, i*KL*128 + kk*128 + p]
    i2ta = input2.T.reshape(NCORES, KL, P, B) if V2 else None
    maps = []
    for i in range(NCORES):
        sl = slice(i * BL, (i + 1) * BL)
        x_sh = np.ascontiguousarray(input1[sl].reshape(BL, C1, HW))
        if V2:
            wts = np.ascontiguousarray(wt[i * KL * P:(i + 1) * KL * P])
            in2ta = np.ascontiguousarray(
                i2ta[i].transpose(1, 0, 2).reshape(P, KL * B)
            )
            maps.append({"x": x_sh, "wts": wts, "in2ta": in2ta, "fcb": fcb})
        else:
            # in2t[p, k*BL + b] = input2[i*BL + b, k*128 + p]
            i2t = np.ascontiguousarray(
                input2[sl].T.reshape(KC, P, BL).transpose(1, 0, 2).reshape(P, KC * BL)
            )
            maps.append({"x": x_sh, "wt": wt, "in2t": i2t, "fcb": fcb})
    return maps


def _assemble(results):
    outs = []
    for i in range(NCORES):
        arr = np.asarray(results[i]["out"])                 # [128, BL*CO]
        # arr[ci, b*CO + co] = g[b, co*128 + ci]
        outs.append(
            arr.reshape(P, BL, CO).transpose(1, 2, 0).reshape(BL, C1)
        )
    return np.ascontiguousarray(np.concatenate(outs, axis=0).astype(np.float32))


def run(input1, input2, fc_w, fc_b, trace=False, **trace_kwargs):
    nc = _get_nc()
    res = run_bass_kernel_spmd(
        nc,
        _in_maps(input1, input2, fc_w, fc_b),
        core_ids=list(range(NCORES)),
        trace=trace,
        **trace_kwargs,
    )
    return _assemble(res.results), res


def kernel(input1, input2, fc_w, fc_b):
    global V2
    try:
        out, _ = run(input1, input2, fc_w, fc_b)
        return out
    except Exception:
        if not V2:
            raise
        # collective path failed in this environment; fall back to the
        # replicated-weights variant (no cross-core communication)
        V2 = False
        out, _ = run(input1, input2, fc_w, fc_b)
        return out

